# revision 1
# baseline (speedup 1.0000x reference)
"""DeepForest (nn_DeepForest_30356828848186) Trainium2 Bass kernel.

Strategy: data-parallel over batch across the 8 NeuronCores (B=8192 ->
1024 rows/core, forest parameters replicated) -- no collectives needed.

Per core:
  * Nodes (t, m) are linearized level-major (slot = (m-1)*20 + t), padded
    620 -> 640 slots = 320 pairs; each pair packs 2 nodes x 64 hidden units
    onto the 128 PE partitions.
  * matmul1 computes a = x @ W1dense per pair into PSUM [128, 1024], where
    W1dense is the per-node feature-scattered W1 with |W2| folded in
    (w*relu(y) = sign(w)*relu(|w|*y)).  Run as 3 error-compensated f32r
    (TF32) passes: Wr@xr + Wr@xl + Wl@xr with hi/lo tf32 splits done on the
    host -- full-rate PE with fp32-level accuracy (residual ~2^-22).
  * relu+bias via ScalarE activation (per-partition bias), producing both
    an f32r-rounded hr and fp32 h; hl = h - hr (f32r) on VectorE.
  * z = sum_h sign(W2)*relu(...) via sign-matrix matmuls accumulated 64
    pairs into PSUM [128 slots, 512 B] (z = sgn@hr + sgn@hl, exact).
  * gate bits s = (z > -b2) via per-partition threshold compare.
  * PE-transpose s into [batch, slot] layout, then the tree walk as mask
    algebra (mask_left = mask*s, mask_right = mask - mask_left per level),
    leaf values, votes over 20 trees, and first-tie argmax -- all on DVE.

Self-contained: hardcodes all shapes; only needs numpy + concourse (bass).
"""
import os
import sys

import numpy as np

sys.path.insert(0, "/opt/trn_rl_repo")

import concourse.bass as bass  # noqa: E402
import concourse.bacc as bacc  # noqa: E402
import concourse.mybir as mybir  # noqa: E402
from concourse.tile import TileContext  # noqa: E402
from concourse import bass_utils  # noqa: E402

F32 = mybir.dt.float32
F32R = mybir.dt.float32r
AX = mybir.AxisListType
OP = mybir.AluOpType
ACT_RELU = mybir.ActivationFunctionType.Relu

T, M, D, B, F, H = 20, 31, 5, 8192, 256, 64
L = 16
NSLOT = 640          # padded node slots (620 real)
NP = NSLOT // 2      # 320 node pairs
NZ = NSLOT // 128    # 5 z-tiles
NCORE = 8
BLOC = B // NCORE    # 1024
NCLS = 10


def _tf32_round(a: np.ndarray) -> np.ndarray:
    bits = a.astype(np.float32).view(np.uint32)
    bits = bits + np.uint32(0x1000)
    bits &= np.uint32(0xFFFFE000)
    return bits.view(np.float32)


def _pack_inputs(x, W1, b1, W2, b2, feats, best_left, best_right):
    x = np.asarray(x, np.float32)
    W1 = np.asarray(W1, np.float32)
    b1 = np.asarray(b1, np.float32)
    W2 = np.asarray(W2, np.float32)
    b2 = np.asarray(b2, np.float32)
    feats = np.asarray(feats)
    bl = np.asarray(best_left).astype(np.float32)
    br = np.asarray(best_right).astype(np.float32)

    absW2 = np.abs(W2)
    sgnW2 = np.sign(W2).astype(np.float32)

    W1f = np.zeros((T, M, F, H), np.float32)
    ti = np.arange(T)[:, None, None]
    mi = np.arange(M)[None, :, None]
    W1f[ti, mi, feats] = W1 * absW2[:, :, None, :]
    b1f = b1 * absW2

    slot_t = np.arange(620) % 20
    slot_m = np.arange(620) // 20 + 1

    def node_w(slot):
        if slot >= 620:
            return np.zeros((F, H), np.float32)
        return W1f[slot_t[slot], slot_m[slot] - 1]

    def node_b1(slot):
        if slot >= 620:
            return np.zeros(H, np.float32)
        return b1f[slot_t[slot], slot_m[slot] - 1]

    def node_sgn(slot):
        if slot >= 620:
            return np.zeros(H, np.float32)
        return sgnW2[slot_t[slot], slot_m[slot] - 1]

    ncols = 640
    wk = np.zeros((NP, 128, ncols), np.float32)
    b1c = np.zeros((128, NP), np.float32)
    for u in range(NP):
        a_, b_ = 2 * u, 2 * u + 1
        wa = np.concatenate([node_w(a_), node_w(b_)], axis=1)
        war = _tf32_round(wa)
        wal = _tf32_round(wa - war)
        wk[u, :, 0:128] = war[0:128]
        wk[u, :, 128:256] = war[128:256]
        wk[u, :, 256:384] = wal[0:128]
        wk[u, :, 384:512] = wal[128:256]
        b1c[0:64, u] = node_b1(a_)
        b1c[64:128, u] = node_b1(b_)
        ca, cb = (2 * u) % 128, (2 * u + 1) % 128
        wk[u, 0:64, 512 + ca] = node_sgn(a_)
        wk[u, 64:128, 512 + cb] = node_sgn(b_)

    thr = np.full((128, NZ), 1e30, np.float32)
    for s in range(620):
        thr[s % 128, s // 128] = -b2[slot_t[s], slot_m[s] - 1]

    blbr = np.zeros((128, 320), np.float32)
    brc = np.zeros((128, 320), np.float32)
    for pos in range(L):
        for t in range(T):
            blbr[:, pos * 20 + t] = bl[t, pos] - br[t, pos]
            brc[:, pos * 20 + t] = br[t, pos]

    wvec = np.zeros((128, NCLS), np.float32)
    for c in range(NCLS):
        wvec[:, c] = float(NCLS - c)

    ident = np.eye(128, dtype=np.float32)
    consts = np.concatenate(
        [b1c, np.pad(thr, ((0, 0), (0, 3))), blbr, brc,
         np.pad(wvec, ((0, 0), (0, 6))), ident], axis=1)

    shared = {"wk": wk, "consts": consts}

    xts = []
    for c in range(NCORE):
        xs = x[c * BLOC:(c + 1) * BLOC]
        xt = np.ascontiguousarray(xs.T).reshape(2, 128, BLOC)
        xtr = _tf32_round(xt)
        xtl = _tf32_round(xt - xtr)
        xts.append(np.concatenate([xtr, xtl], axis=0))
    return shared, xts


def _build(nc: bass.Bass):
    nxt = 4
    ncols = 640
    NCC = NP + 8 + 320 + 320 + 16 + 128
    xt_d = nc.dram_tensor("xt", [nxt, 128, BLOC], F32, kind="ExternalInput").ap()
    wk_d = nc.dram_tensor("wk", [NP, 128, ncols], F32, kind="ExternalInput").ap()
    cc_d = nc.dram_tensor("consts", [128, NCC], F32, kind="ExternalInput").ap()
    out_d = nc.dram_tensor("out", [BLOC], F32, kind="ExternalOutput").ap()

    with TileContext(nc) as tc:
        with tc.tile_pool(name="const", bufs=1) as cp:
            xall = cp.tile([128, nxt * BLOC], F32R, name="xall")
            nc.sync.dma_start(xall[:].rearrange("f (k b) -> f k b", k=nxt),
                              xt_d.bitcast(F32R).rearrange("k f b -> f k b"))
            xts = [xall[:, k * BLOC:(k + 1) * BLOC] for k in range(nxt)]
            cc = cp.tile([128, NCC], F32, name="cc")
            nc.sync.dma_start(cc[:], cc_d)
            o = 0
            b1c = cc[:, o:o + NP]; o += NP
            thr = cc[:, o:o + NZ]; o += 8
            blbr = cc[:, o:o + 320]; o += 320
            brc = cc[:, o:o + 320]; o += 320
            wvec = cc[:, o:o + NCLS]; o += 16
            ident = cc[:, o:o + 128]; o += 128
            s_all = cp.tile([128, NZ * BLOC], F32)
            out_sb = cp.tile([128, BLOC // 128], F32)
            tc.strict_bb_all_engine_barrier()

            terms = [(0, 0), (0, 2), (1, 1), (1, 3), (2, 0), (3, 1)]
            with tc.tile_pool(name="wp", bufs=6) as wp, \
                 tc.tile_pool(name="hp", bufs=6) as hp, \
                 tc.tile_pool(name="php", bufs=2, space="PSUM") as php, \
                 tc.tile_pool(name="pzp", bufs=4, space="PSUM") as pzp:
                pz = {}
                for u in range(NP):
                    zt, first, last = u // 64, u % 64 == 0, u % 64 == 63
                    wt = wp.tile([128, ncols], F32R)
                    nc.sync.dma_start(wt[:], wk_d[u].bitcast(F32R))
                    ph = php.tile([128, BLOC], F32)
                    for i, (wb, xb) in enumerate(terms):
                        for c in range(2):
                            cs = slice(c * 512, (c + 1) * 512)
                            nc.tensor.matmul(
                                ph[:, cs], wt[:, wb * 128:(wb + 1) * 128],
                                xts[xb][:, cs],
                                start=(i == 0), stop=(i == len(terms) - 1))
                    sg = wt[:, ncols - 128:ncols]
                    if first:
                        pz[zt] = [pzp.tile([128, 512], F32, name=f"pz{zt}_{c}",
                                           tag="pz")
                                  for c in range(2)]
                    hr = hp.tile([128, BLOC], F32R, name="hr")
                    nc.scalar.activation(hr[:], ph[:], ACT_RELU,
                                         bias=b1c[:, u:u + 1], scale=1.0)
                    h = hp.tile([128, BLOC], F32, name="h")
                    nc.scalar.activation(h[:], ph[:], ACT_RELU,
                                         bias=b1c[:, u:u + 1], scale=1.0)
                    hl = hp.tile([128, BLOC], F32R, name="hl")
                    nc.vector.tensor_tensor(hl[:], h[:], hr[:], OP.subtract)
                    for c in range(2):
                        cs = slice(c * 512, (c + 1) * 512)
                        nc.tensor.matmul(pz[zt][c][:], sg, hr[:, cs],
                                         start=first, stop=False)
                        nc.tensor.matmul(pz[zt][c][:], sg, hl[:, cs],
                                         start=False, stop=last)
                    if last:
                        for c in range(2):
                            dst = s_all[:, zt * BLOC + c * 512:
                                        zt * BLOC + (c + 1) * 512]
                            nc.vector.tensor_scalar(dst, pz[zt][c][:],
                                                    thr[:, zt:zt + 1], None,
                                                    OP.is_gt)

            with tc.tile_pool(name="ptp", bufs=2, space="PSUM") as ptp, \
                 tc.tile_pool(name="stp", bufs=2) as stp, \
                 tc.tile_pool(name="wkp", bufs=2) as wkp:
                for bt in range(BLOC // 128):
                    st = stp.tile([128, NSLOT], F32)
                    for zt in range(NZ):
                        pt = ptp.tile([128, 128], F32)
                        nc.tensor.transpose(
                            pt[:],
                            s_all[:, zt * BLOC + bt * 128:
                                  zt * BLOC + (bt + 1) * 128],
                            ident[:])
                        nc.vector.tensor_copy(st[:, zt * 128:(zt + 1) * 128],
                                              pt[:])

                    m1 = wkp.tile([128, 40], F32, name="m1")
                    nc.vector.tensor_copy(m1[:, 0:20], st[:, 0:20])
                    nc.vector.tensor_scalar(m1[:, 20:40], st[:, 0:20],
                                            -1.0, 1.0, OP.mult, OP.add)
                    mprev = m1
                    for k in range(1, 4):
                        nq = 2 ** k
                        off = (nq - 1) * 20
                        mn = wkp.tile([128, nq * 40], F32, name=f"m{k + 1}")
                        mn3 = mn[:].rearrange("p (q e t) -> p q e t", e=2, t=20)
                        sv = st[:, off:off + nq * 20].rearrange(
                            "p (q t) -> p q t", t=20)
                        mv = mprev[:].rearrange("p (q t) -> p q t", t=20)
                        nc.vector.tensor_tensor(mn3[:, :, 0, :], mv, sv, OP.mult)
                        nc.vector.tensor_tensor(mn3[:, :, 1, :], mv,
                                                mn3[:, :, 0, :], OP.subtract)
                        mprev = mn
                    val = wkp.tile([128, 320], F32, name="val")
                    nc.vector.tensor_tensor(val[:], st[:, 300:620], blbr[:],
                                            OP.mult)
                    nc.vector.tensor_tensor(val[:], val[:], brc[:], OP.add)
                    prod = wkp.tile([128, 320], F32, name="prod")
                    nc.vector.tensor_tensor(prod[:], mprev[:], val[:], OP.mult)
                    pred = wkp.tile([128, 20], F32, name="pred")
                    nc.vector.tensor_reduce(
                        pred[:], prod[:].rearrange("p (q t) -> p t q", t=20),
                        axis=AX.X, op=OP.add)
                    counts = wkp.tile([128, NCLS], F32, name="counts")
                    eqt = wkp.tile([128, 20], F32, name="eqt")
                    for cls in range(NCLS):
                        nc.vector.tensor_scalar(eqt[:], pred[:], float(cls),
                                                None, OP.is_equal, OP.add,
                                                accum_out=counts[:, cls:cls + 1])
                    cmax = wkp.tile([128, 1], F32, name="cmax")
                    nc.vector.tensor_reduce(cmax[:], counts[:], axis=AX.X,
                                            op=OP.max)
                    pick = wkp.tile([128, NCLS], F32, name="pick")
                    nc.vector.tensor_scalar(pick[:], counts[:], cmax[:], None,
                                            OP.is_equal)
                    nc.vector.tensor_tensor(pick[:], pick[:], wvec[:], OP.mult)
                    mv_ = wkp.tile([128, 1], F32, name="mv_")
                    nc.vector.tensor_reduce(mv_[:], pick[:], axis=AX.X,
                                            op=OP.max)
                    nc.vector.tensor_scalar(out_sb[:, bt:bt + 1], mv_[:],
                                            -1.0, float(NCLS), OP.mult, OP.add)

            nc.sync.dma_start(out_d.rearrange("(b p) -> p b", p=128), out_sb[:])
    return nc


_CACHE = {}


def kernel(x, W1, b1, W2, b2, feats, best_left, best_right) -> np.ndarray:
    shared, xts = _pack_inputs(x, W1, b1, W2, b2, feats, best_left, best_right)
    if "nc" not in _CACHE:
        nc = bacc.Bacc("TRN2", target_bir_lowering=False, debug=False,
                       num_devices=NCORE)
        _build(nc)
        nc.compile()
        _CACHE["nc"] = nc
    nc = _CACHE["nc"]
    in_maps = [dict(shared, xt=xts[c]) for c in range(NCORE)]
    res = bass_utils.run_bass_kernel_spmd(nc, in_maps,
                                          core_ids=list(range(NCORE)))
    out = np.concatenate([res.results[c]["out"] for c in range(NCORE)])
    return out.astype(np.float32)

